# revision 39
# baseline (speedup 1.0000x reference)
"""Multi-head causal self-attention (B=2, T=2048, D=2048, H=16) on 8 Trainium2
NeuronCores.

Sharding: core c handles batch b = c//4 and 4 heads hs = 4*(c%4) .. hs+4
(batch x tensor-parallel heads). Each core computes Q/K/V projections for its
head slice, causal attention for its 4 heads, and a row-parallel partial of the
output projection (out_heads_slice @ wo_slice.T). The 4 partials per batch are
summed on the host (wo row-parallel reduce); bo is added on the host during the
gather.

Single fused pipeline per 512-row t-tile (t4): the x chunks are loaded ONCE and
shared by the Q, K and V projections (the old kernel loaded x twice). The
attention work for tile t4 (scores -> exp -> P~V -> transpose -> output-proj
partial) is EMISSION-INTERLEAVED with the projections of tile t4+1 so the PE
never drains while the scalar engine runs the exps.

Device layout notes (all matmuls contract over the partition dim, out = lhsT.T @ rhs):
 - x is fed pre-transposed per batch: xT [D, T].
 - Q, K are produced transposed: QT/KT [hd, t] (hd on partitions); scores are
   computed transposed as ST[k, q]; softmax runs WITHOUT max subtraction
   (scores are O(10), exp is safe in f32) and the row sums come for free as an
   extra ones-column appended to V in the P~V matmul.
 - V is produced in natural [t, hd] layout straight into VP [t, hd | 1].
 - P~ = exp(scale * ST) is masked only on diagonal 128x128 blocks; blocks
   entirely above the causal diagonal are never computed.
 - O = P~ @ V' lands as [q, hd | rowsum]; normalize by the reciprocal of the
   rowsum column, PE-transpose to OT [hd, q] for the final projection.

DMA queues: x tiles on SP/HWDGE (cold-start groups fanned across SP/Act/Pool —
the 16 HW DMA engines transfer in parallel, so issue bandwidth is what
matters), weights on Act/HWDGE in consumption order, mask/identity/v-bias and
the output stores on Pool/SWDGE (the last tile's stores on the by-then-idle SP
queue to shorten the drain).

Projections and the output projection run in fp8-e4m3 with
perf_mode=DoubleRow (two 128-contraction chunks packed per PE cell, ~1.44x+
over bf16 at FD=512). Weights are host-scaled x64 so they sit in the e4m3
normal range; Q/K biases carry the same x64 and the exp scale absorbs the
resulting 64^2 on the score path.

The V bias is never added on device: VP holds 64*(v - bv) = x@wv and the
softmax gives attn - bv exactly (Sum p (v-bv) / Sum p). That mean-subtraction
is what makes the fp8 output projection accurate: attn output is dominated by
the bv mean, and quantizing only the fluctuation part keeps the e4m3 noise
small. The ones-column is 4.0 (= 64/16) so the normalize yields 16*(attn-bv),
putting osb/ot in good e4m3 range; wo is x64 e4m3, so the stored f16 partial
is 1024x the true partial. The host divides by 1024 and adds bv@wo.T + bo in
f64. Scores and PV stay bf16 (PSUM accumulation always f32).
"""

import sys
import numpy as np

if '/opt/trn_rl_repo' not in sys.path:
    sys.path.insert(0, '/opt/trn_rl_repo')

import ml_dtypes
from contextlib import ExitStack

import concourse.mybir as mybir
import concourse.tile as tile
from concourse import bacc
from concourse.bass_utils import run_bass_kernel_spmd

B, T, D, H = 2, 2048, 2048, 16
HD = 128           # head dim
P = 128            # partitions
HPC = 4            # heads per core
NCORES = 8
SCALE = float(HD) ** -0.5
DC = D // P        # 16 contraction chunks for projections
NT = T // P        # 16 t-chunks of 128
QT_TILES = T // 512  # 4 t-tiles of 512

BF16 = mybir.dt.bfloat16
F32 = mybir.dt.float32
F16 = mybir.dt.float16
F8E4 = mybir.dt.float8e4
NPBF16 = ml_dtypes.bfloat16
NPF8E4 = ml_dtypes.float8_e4m3
WSCALE = 64.0          # host pre-scale on wq/wk/wv (+ biases) for fp8 range
DR = mybir.MatmulPerfMode.DoubleRow

_BUILD_CACHE = {}


def _merge(primary, secondary):
    """Interleave two thunk lists proportionally, preserving each list's
    internal order. Primary items sort first on ties."""
    items = []
    for li, lst in ((0, primary), (1, secondary)):
        n = len(lst)
        for i, fn in enumerate(lst):
            items.append(((i + 0.5) / n, li, i, fn))
    items.sort(key=lambda t: (t[0], t[1], t[2]))
    return [fn for _, _, _, fn in items]


def _weave(primary, secondary):
    for fn in _merge(primary, secondary):
        fn()


def _build(causal: bool):
    """Build the per-core Bass program (identical across cores; data differs)."""
    nc = bacc.Bacc("TRN2", target_bir_lowering=False, debug=False)

    xT = nc.dram_tensor("xT", [D, T], F8E4, kind="ExternalInput").ap()
    wqT = nc.dram_tensor("wqT", [D, HPC * HD], F8E4, kind="ExternalInput").ap()
    wkT = nc.dram_tensor("wkT", [D, HPC * HD], F8E4, kind="ExternalInput").ap()
    wvT = nc.dram_tensor("wvT", [D, HPC * HD], F8E4, kind="ExternalInput").ap()
    woT = nc.dram_tensor("woT", [HPC * HD, D], F8E4, kind="ExternalInput").ap()
    bq = nc.dram_tensor("bq", [P, HPC], F32, kind="ExternalInput").ap()
    bk = nc.dram_tensor("bk", [P, HPC], F32, kind="ExternalInput").ap()
    tri = nc.dram_tensor("tri", [P, P], BF16, kind="ExternalInput").ap()
    ident = nc.dram_tensor("ident", [P, P], BF16, kind="ExternalInput").ap()
    out = nc.dram_tensor("out", [T, D], F16, kind="ExternalOutput").ap()

    with tile.TileContext(nc) as tc:
        with ExitStack() as ctx:
            persist = ctx.enter_context(tc.tile_pool(name="persist", bufs=1))

            wq_sb = persist.tile([P, DC, HPC * HD], F8E4, name="wq_sb")
            wk_sb = persist.tile([P, DC, HPC * HD], F8E4, name="wk_sb")
            wv_sb = persist.tile([P, DC, HPC * HD], F8E4, name="wv_sb")
            wo_sb = persist.tile([P, HPC, D], F8E4, name="wo_sb")
            bq_sb = persist.tile([P, HPC], F32, name="bq_sb")
            bk_sb = persist.tile([P, HPC], F32, name="bk_sb")
            tri_sb = persist.tile([P, P], BF16, name="tri_sb")
            id_sb = persist.tile([P, P], BF16, name="id_sb")
            KT_sb = persist.tile([P, HPC, T], BF16, name="KT_sb")
            # V' with ones column: [t-within-chunk, head, t-chunk, hd+1]
            VP_sb = persist.tile([P, HPC, NT, HD + 1], BF16, name="VP_sb")

            # pools
            # 8 bufs: tile t4's 4 group tiles + all 4 prefetched t4+1 groups
            # may be alive at once — fewer bufs stalls the prefetch DMA on
            # pool reuse and the V chains then wait on x mid-tile
            ax = ctx.enter_context(tc.tile_pool(name="ax", bufs=8))
            aqt = ctx.enter_context(tc.tile_pool(name="aqt", bufs=2))
            aot = ctx.enter_context(tc.tile_pool(name="aot", bufs=3))
            cpt = ctx.enter_context(tc.tile_pool(name="cpt", bufs=48))
            csm = ctx.enter_context(tc.tile_pool(name="csm", bufs=8))
            cob = ctx.enter_context(tc.tile_pool(name="cob", bufs=6))
            ps = ctx.enter_context(tc.tile_pool(name="ps", bufs=1, space="PSUM"))

            def load_x(t4):
                """Group-DMAs of 4 chunks each on SP/HWDGE; returns per-PAIR
                AP views [P, 2, 512] indexed by dp (8 chunk-pairs)."""
                xas = []
                for g in range(4):
                    xg = ax.tile([P, 4, 512], F8E4, tag="xa", name=f"xa{t4}_{g}")
                    # tile 0 cold start: both HWDGE queues stream x in Q-chain
                    # consumption order (group 0 as 128KB pairs so the first
                    # matmul starts ASAP); the SW Pool queue is too slow here.
                    if t4 == 0 and g == 0:
                        for c in range(2):
                            nc.scalar.dma_start(
                                xg[:, 2 * c:2 * c + 2, :],
                                xT[256 * c:256 * (c + 1), 0:512].rearrange(
                                    "(c p) n -> p c n", p=P))
                    else:
                        eng = nc.scalar if (t4 == 0 and g == 1) else nc.sync
                        eng.dma_start(
                            xg[:], xT[g * 512:(g + 1) * 512,
                                      t4 * 512:(t4 + 1) * 512].rearrange(
                                          "(c p) n -> p c n", p=P))
                    xas.extend(xg[:, 2 * c:2 * c + 2, :] for c in range(2))
                return xas

            NDP = DC // 2  # 8 fp8 DoubleRow chunk-pairs per projection

            def proj_items(t4, xas, qtile):
                """24 thunks: Q(8 d-pairs), K(8), V(8). Tag 'p' PSUM.
                All matmuls fp8-e4m3 perf_mode=DoubleRow (contraction 256)."""
                psq = [None] * HPC
                psk = [None] * HPC
                psv = [None] * HPC

                def q_chunk(dp):
                    def fn():
                        if dp == 0:
                            for h in range(HPC):
                                psq[h] = ps.tile([P, 512], F32, tag="p", bufs=4,
                                                 name=f"psq{t4}_{h}")
                        for h in range(HPC):
                            nc.tensor.matmul(psq[h][:],
                                             wq_sb[:, 2 * dp:2 * dp + 2, h * HD:(h + 1) * HD],
                                             xas[dp][:], start=(dp == 0),
                                             stop=(dp == NDP - 1), perf_mode=DR)
                        if dp == NDP - 1:
                            for h in range(HPC):
                                nc.vector.tensor_scalar_add(qtile[:, h, :], psq[h][:],
                                                            bq_sb[:, h:h + 1])
                    return fn

                def k_chunk(dp):
                    def fn():
                        if dp == 0:
                            for h in range(HPC):
                                psk[h] = ps.tile([P, 512], F32, tag="p", bufs=4,
                                                 name=f"psk{t4}_{h}")
                        for h in range(HPC):
                            nc.tensor.matmul(psk[h][:],
                                             wk_sb[:, 2 * dp:2 * dp + 2, h * HD:(h + 1) * HD],
                                             xas[dp][:], start=(dp == 0),
                                             stop=(dp == NDP - 1), perf_mode=DR)
                        if dp == NDP - 1:
                            for h in range(HPC):
                                nc.vector.tensor_scalar_add(
                                    KT_sb[:, h, t4 * 512:(t4 + 1) * 512], psk[h][:],
                                    bk_sb[:, h:h + 1])
                    return fn

                def v_half(j, half):
                    # V projection in natural [t, hd] layout: x chunk cols are
                    # the stationary operand, all 4 heads' weights move. One
                    # full chain per t-chunk j (not d-interleaved) so chunk
                    # j's PSUM bank frees early for the next tile's Q chains.
                    def fn():
                        if half == 0:
                            psv[j] = ps.tile([P, 512], F32, tag="p", bufs=4,
                                             name=f"psv{t4}_{j}")
                        for dp in range(4 * half, 4 * half + 4):
                            nc.tensor.matmul(psv[j][:],
                                             xas[dp][:, :, j * P:(j + 1) * P],
                                             wv_sb[:, 2 * dp:2 * dp + 2, :],
                                             start=(dp == 0), stop=(dp == NDP - 1),
                                             perf_mode=DR)
                        if half == 1:
                            kb = 4 * t4 + j
                            # no bias: VP holds 64*(v - bv) so the softmax
                            # output is mean-subtracted for the fp8 out-proj
                            for h in range(HPC):
                                nc.vector.tensor_copy(
                                    out=VP_sb[:, h, kb, 0:HD],
                                    in_=psv[j][:, h * HD:(h + 1) * HD])
                    return fn

                return ([q_chunk(dp) for dp in range(NDP)]
                        + [k_chunk(dp) for dp in range(NDP)]
                        + [v_half(j, half) for j in range(HPC) for half in (0, 1)])

            def make_attn(t4, qtile, ot_tile, pending_fins):
                """Attention for tile t4: per head scores->exp->mask, P~V
                chains with staggered PE transposes, and (after head 3) the
                output-projection partials + store. Head h+1's scores are
                emitted between head h's chains so the scalar engine's exps
                stay ahead of the PE."""
                kmax = 4 * t4 + 4 if causal else NT
                pts = [[None] * kmax for _ in range(HPC)]
                osbs = [[None] * HPC for _ in range(HPC)]

                def score_block(h, kb):
                    def fn():
                        qoff = max(0, kb - 4 * t4) * P if causal else 0
                        w = 512 - qoff
                        stp = ps.tile([P, 512], F32, tag="sf", bufs=2,
                                      name=f"st{t4}_{h}_{kb}")
                        nc.tensor.matmul(stp[:, 0:w], KT_sb[:, h, kb * P:(kb + 1) * P],
                                         qtile[:, h, qoff:512], start=True, stop=True)
                        pt = cpt.tile([P, 512], BF16, tag="pt", name=f"pt{t4}_{h}_{kb}")
                        nc.scalar.activation(pt[:, 0:w], stp[:, 0:w],
                                             mybir.ActivationFunctionType.Exp,
                                             scale=SCALE / (WSCALE * WSCALE))
                        if causal and kb >= 4 * t4:
                            nc.vector.tensor_mul(out=pt[:, 0:P], in0=pt[:, 0:P], in1=tri_sb[:])
                        pts[h][kb] = pt
                    return fn

                def pv_chain(h, qs):
                    def fn():
                        qb = 4 * t4 + qs
                        klim = qb + 1 if causal else NT
                        ops = ps.tile([P, HD + 1], F32, tag="o", bufs=2,
                                      name=f"o{t4}_{h}_{qs}")
                        for kb in range(klim):
                            qoff = max(0, kb - 4 * t4) * P if causal else 0
                            c0 = qs * P - qoff
                            nc.tensor.matmul(ops[:], pts[h][kb][:, c0:c0 + P],
                                             VP_sb[:, h, kb, :],
                                             start=(kb == 0), stop=(kb == klim - 1))
                        rec = csm.tile([P, 1], F32, tag="rec", name=f"rec{t4}_{h}_{qs}")
                        nc.vector.reciprocal(rec[:], ops[:, HD:HD + 1])
                        # ones-col is 4.0 = 64/16, so this is 16*(attn - bv):
                        # mean-subtracted and scaled into e4m3 range
                        osb = csm.tile([P, HD], BF16, tag="osb", name=f"osb{t4}_{h}_{qs}")
                        nc.vector.tensor_scalar_mul(osb[:], ops[:, 0:HD], rec[:])
                        osbs[h][qs] = osb
                    return fn

                def o_transpose(h, qs):
                    def fn():
                        tp2 = ps.tile([P, P], BF16, tag="o", bufs=2,
                                      name=f"tpo{t4}_{h}_{qs}")
                        nc.tensor.transpose(tp2[:], osbs[h][qs][:], id_sb[:])
                        # DVE cast bf16 -> e4m3 for the DoubleRow out-proj
                        nc.vector.tensor_copy(out=ot_tile[:, h, qs, :], in_=tp2[:])
                    return fn

                def fin(qs, n):
                    def fn():
                        tch = 4 * t4 + qs
                        # alternate PSUM tags so fins don't monopolize the
                        # score stream's two "sf" banks
                        fp = ps.tile([P, 512], F32, tag=("sf" if n % 2 else "o"),
                                     bufs=2, name=f"fin{t4}_{qs}_{n}")
                        for hh in range(0, HPC, 2):
                            nc.tensor.matmul(fp[:], ot_tile[:, hh:hh + 2, qs, :],
                                             wo_sb[:, hh:hh + 2, n * 512:(n + 1) * 512],
                                             start=(hh == 0), stop=(hh == HPC - 2),
                                             perf_mode=DR)
                        ob = cob.tile([P, 512], F16, tag="ob", name=f"ob{t4}_{qs}_{n}")
                        # last tile: exps are done, so the scalar engine is
                        # free — split the f32->f16 casts across DVE and ACT
                        # so the drain doesn't serialize on one engine
                        if t4 == QT_TILES - 1 and n % 2 == 0:
                            nc.scalar.copy(out=ob[:], in_=fp[:])
                        else:
                            nc.vector.tensor_copy(out=ob[:], in_=fp[:])
                        # all stores ride the SP HWDGE queue: the Pool SWDGE
                        # queue is too slow for 8MB of output (end-of-run
                        # backlog), and issuing from the Act engine steals
                        # issue slots from the exps. The sync engine is idle.
                        eng = nc.sync
                        eng.dma_start(out[tch * P:(tch + 1) * P,
                                          n * 512:(n + 1) * 512], ob[:])
                    return fn

                def pv_block(h):
                    # P~V chains with the transpose of chunk qs emitted after
                    # the NEXT chain so the PE never waits on the DVE
                    # normalize.
                    its = []
                    for qs in range(4):
                        its.append(pv_chain(h, qs))
                        if qs >= 1:
                            its.append(o_transpose(h, qs - 1))
                    its.append(o_transpose(h, 3))
                    return its

                # The tile's own output-projection partials are returned
                # separately and woven into the NEXT tile's Act-paced score
                # sections (where the PE would otherwise idle behind the
                # exps); the previous tile's partials arrive here as
                # pending_fins.
                #
                # Heads 0-1's OFF-DIAGONAL scores (kb < 4*t4, which need only
                # QT(t4) and older KT) are returned separately so they can run
                # inside tile t4's own K/V projection window — that pulls
                # ~14us of exp work off the exp-bound attention tail.
                off_heads = (0, 1) if causal and t4 > 0 else ()
                off_items = [score_block(h, kb)
                             for h in off_heads for kb in range(4 * t4)]
                fins = [fin(qs, n) for qs in range(4) for n in range(4)]
                nf = len(pending_fins)
                cuts = [0, nf * 1 // 10, nf * 2 // 10, nf * 6 // 10, nf]
                items = []
                for h in range(HPC):
                    k0 = 4 * t4 if h in off_heads else 0
                    filler = list(pending_fins[cuts[h]:cuts[h + 1]])
                    if h >= 1:
                        filler = _merge(pv_block(h - 1), filler)
                    items.extend(_merge(
                        [score_block(h, kb) for kb in range(k0, kmax)], filler))
                if t4 == QT_TILES - 1:
                    # last tile: the final head's transposes go right after
                    # their chains (PE briefly waits on the DVE normalize, but
                    # that frees fin(qs,*) immediately) and the fins follow so
                    # their casts/stores drain while later chains still run
                    h = HPC - 1
                    for qs in range(4):
                        items.append(pv_chain(h, qs))
                        items.append(o_transpose(h, qs))
                        items.extend(fins[4 * qs:4 * qs + 4])
                    fins = []
                else:
                    items.extend(pv_block(HPC - 1))
                return off_items, items, fins

            # ---- initial DMAs, spread across issue queues ----
            # HWDGE issue slots are the cold-start bottleneck (~1.25us per
            # DMA per queue), so bulk loads go as 4-chunk group DMAs via
            # einops views, split across the SP and Act queues with the
            # first Q matmul's dependencies (wq group 0 on SP, x group 0 on
            # Act) issued first on each.
            def wview(w, g):
                return w[g * 512:(g + 1) * 512, :].rearrange(
                    "(c p) n -> p c n", p=P)

            # first matmul needs only wq pair 0 + x pair 0: issue those as
            # 128KB pair-DMAs so they land ahead of the 3MB weight stream
            def wpair(w, p):
                return w[p * 256:(p + 1) * 256, :].rearrange(
                    "(c p) n -> p c n", p=P)

            nc.sync.dma_start(wq_sb[:, 0:2, :], wpair(wqT, 0))
            nc.sync.dma_start(wq_sb[:, 2:4, :], wpair(wqT, 1))
            xtiles = {0: load_x(0)}
            for g in range(1, 4):
                nc.sync.dma_start(wq_sb[:, 4 * g:4 * g + 4, :], wview(wqT, g))
            nc.sync.dma_start(bq_sb[:], bq[:])
            nc.sync.dma_start(bk_sb[:], bk[:])
            # non-critical loads go behind the x groups on the Pool queue so
            # they don't steal HBM bandwidth from the cold-start x/wq stream
            # (wk is first needed ~10us in, wv ~15us, wo ~45us)
            nc.gpsimd.dma_start(tri_sb[:], tri[:])
            nc.gpsimd.dma_start(id_sb[:], ident[:])
            for g in range(4):
                nc.gpsimd.dma_start(wk_sb[:, 4 * g:4 * g + 4, :], wview(wkT, g))
            for g in range(4):
                nc.gpsimd.dma_start(wv_sb[:, 4 * g:4 * g + 4, :], wview(wvT, g))
            for hh in range(HPC):
                nc.gpsimd.dma_start(wo_sb[:, hh, :], woT[hh * P:(hh + 1) * P, :])
            # ones column = 64/16: the rowsum keeps the V-path x64 scale down
            # to x4 so the normalize leaves x16 on (attn - bv) for e4m3 range
            nc.gpsimd.memset(VP_sb[:, :, :, HD:HD + 1], WSCALE / 16.0)

            # ---- main pipeline: attention(t4-1) weaves into proj(t4), and
            # the output-projection partials of t4-1 weave into attention(t4)
            # (PE filler for its Act-paced score warm-up) ----
            prev_attn, prev_fins = [], []
            for t4 in range(QT_TILES):
                qtile = aqt.tile([P, HPC, 512], BF16, tag="qt", name=f"qt{t4}")
                ot_tile = aot.tile([P, HPC, 4, P], F8E4, tag="ot", name=f"ot{t4}")
                pitems = proj_items(t4, xtiles[t4], qtile)
                if t4 + 1 < QT_TILES:
                    pitems.insert(0, (lambda n: (lambda: xtiles.__setitem__(
                        n, load_x(n))))(t4 + 1))
                off, items, fins = make_attn(t4, qtile, ot_tile, prev_fins)
                if off:
                    # off-diag scores of THIS tile need QT(t4): confine them
                    # to the K/V portion of the window
                    nq = NDP + (1 if t4 + 1 < QT_TILES else 0)
                    cut = len(prev_attn) * nq // len(pitems)
                    _weave(pitems[:nq], prev_attn[:cut])
                    _weave(pitems[nq:], _merge(prev_attn[cut:], off))
                else:
                    _weave(pitems, prev_attn)
                del xtiles[t4]
                prev_attn, prev_fins = items, fins
            for fn in prev_attn:
                fn()
            for fn in prev_fins:
                fn()

    nc.compile()
    return nc


def _get_program(causal: bool):
    if causal not in _BUILD_CACHE:
        _BUILD_CACHE[causal] = _build(causal)
    return _BUILD_CACHE[causal]


def _prep_in_maps(x, wq, bq, wk, bk, wv, bv, wo, bo):
    # x in e4m3 unscaled (|x| <~ 5.3, fp8 normals reach 2^-6; max 240).
    # Weights x64 so the uniform(+-0.038) range sits in e4m3 normals; the
    # matching x64 goes on the biases, is cancelled by the exp scale (Q,K)
    # and by the 64.0 ones-column (V).
    xf8 = np.asarray(x, dtype=np.float32).astype(NPF8E4)
    tri = np.triu(np.ones((P, P), dtype=np.float32)).astype(NPBF16)
    ident = np.eye(P, dtype=np.float32).astype(NPBF16)
    wqf8 = (np.asarray(wq, dtype=np.float32) * WSCALE).astype(NPF8E4)
    wkf8 = (np.asarray(wk, dtype=np.float32) * WSCALE).astype(NPF8E4)
    wvf8 = (np.asarray(wv, dtype=np.float32) * WSCALE).astype(NPF8E4)
    wof8 = (np.asarray(wo, dtype=np.float32) * WSCALE).astype(NPF8E4)

    in_maps = []
    for c in range(NCORES):
        b = c // 4
        hs = HPC * HD * (c % 4)
        sl = slice(hs, hs + HPC * HD)
        in_maps.append({
            "xT": np.ascontiguousarray(xf8[b].T),
            "wqT": np.ascontiguousarray(wqf8[sl, :].T),
            "wkT": np.ascontiguousarray(wkf8[sl, :].T),
            "wvT": np.ascontiguousarray(wvf8[sl, :].T),
            "woT": np.ascontiguousarray(wof8[:, sl].T),
            "bq": np.ascontiguousarray(
                (np.asarray(bq, np.float32) * WSCALE)[sl].reshape(HPC, P).T),
            "bk": np.ascontiguousarray(
                (np.asarray(bk, np.float32) * WSCALE)[sl].reshape(HPC, P).T),
            "tri": tri,
            "ident": ident,
        })
    return in_maps


def _classify_mask(mask):
    m = np.asarray(mask, dtype=np.float32).reshape(T, T)
    neg = np.isneginf(m)
    if not neg.any():
        return "full"
    if np.array_equal(neg, np.triu(np.ones((T, T), dtype=bool), k=1)):
        return "causal"
    return "other"


def _numpy_reference(x, mask, wq, bq, wk, bk, wv, bv, wo, bo):
    """Fallback for masks that are neither causal nor empty."""
    x = np.asarray(x, np.float32)
    m = np.asarray(mask, np.float32).reshape(T, T)
    q = (x.reshape(-1, D) @ np.asarray(wq, np.float32).T + bq).reshape(B, T, H, HD).transpose(0, 2, 1, 3)
    k = (x.reshape(-1, D) @ np.asarray(wk, np.float32).T + bk).reshape(B, T, H, HD).transpose(0, 2, 1, 3)
    v = (x.reshape(-1, D) @ np.asarray(wv, np.float32).T + bv).reshape(B, T, H, HD).transpose(0, 2, 1, 3)
    outh = np.empty((B, H, T, HD), np.float32)
    negm = np.isneginf(m)
    for b in range(B):
        for h in range(H):
            s = (q[b, h] @ k[b, h].T) * SCALE
            s = np.where(negm, -np.inf, s)
            s = s - s.max(axis=-1, keepdims=True)
            e = np.exp(s)
            p = e / e.sum(axis=-1, keepdims=True)
            outh[b, h] = p @ v[b, h]
    o = outh.transpose(0, 2, 1, 3).reshape(B * T, D)
    return (o @ np.asarray(wo, np.float32).T + bo).reshape(B, T, D).astype(np.float32)


def run_spmd(inputs, trace=False, tmpdir=None):
    """Run the device kernel; returns (output [B,T,D] f32, BassKernelResults)."""
    mode = _classify_mask(inputs["mask"])
    assert mode in ("causal", "full")
    nc = _get_program(mode == "causal")
    in_maps = _prep_in_maps(
        inputs["x"], inputs["wq"], inputs["bq"], inputs["wk"], inputs["bk"],
        inputs["wv"], inputs["bv"], inputs["wo"], inputs["bo"])
    kw = {}
    if trace:
        kw = dict(trace=True, tmpdir=tmpdir)
    # Unprofiled warm-up execution: the first run of a freshly-loaded NEFF
    # measures 5-60us slower (cold device caches); this also pre-populates
    # the jit cache so the measured run below is steady-state.
    try:
        from concourse import bass2jax
        bass2jax.run_bass_via_pjrt(nc, in_maps, n_cores=NCORES)
    except Exception:
        pass
    res = run_bass_kernel_spmd(nc, in_maps, core_ids=list(range(NCORES)), **kw)
    # device partials are 1024*((attn-bv) @ wo.T); add back the (constant)
    # mean row bv @ wo.T and bo here in f64
    bo64 = np.asarray(inputs["bo"], np.float64)
    mean64 = np.asarray(inputs["bv"], np.float64) @ np.asarray(
        inputs["wo"], np.float64).T + bo64
    out = np.empty((B, T, D), np.float32)
    for b in range(B):
        acc = np.zeros((T, D), np.float64)
        for c in range(4 * b, 4 * b + 4):
            acc += res.results[c]["out"].astype(np.float64)
        out[b] = (acc / 1024.0 + mean64).astype(np.float32)
    return out, res


def kernel(**inputs) -> np.ndarray:
    mode = _classify_mask(inputs["mask"])
    if mode == "other":
        return _numpy_reference(**inputs)
    out, _ = run_spmd(inputs)
    return out



# revision 40
# speedup vs baseline: 1.2165x; 1.2165x over previous
"""Multi-head causal self-attention (B=2, T=2048, D=2048, H=16) on 8 Trainium2
NeuronCores.

Sharding: core c handles batch b = c//4 and 4 heads hs = 4*(c%4) .. hs+4
(batch x tensor-parallel heads). Each core computes Q/K/V projections for its
head slice, causal attention for its 4 heads, and a row-parallel partial of the
output projection (out_heads_slice @ wo_slice.T). The 4 partials per batch are
summed on the host (wo row-parallel reduce); bo is added on the host during the
gather.

Single fused pipeline per 512-row t-tile (t4): the x chunks are loaded ONCE and
shared by the Q, K and V projections (the old kernel loaded x twice). The
attention work for tile t4 (scores -> exp -> P~V -> transpose -> output-proj
partial) is EMISSION-INTERLEAVED with the projections of tile t4+1 so the PE
never drains while the scalar engine runs the exps.

Device layout notes (all matmuls contract over the partition dim, out = lhsT.T @ rhs):
 - x is fed pre-transposed per batch: xT [D, T].
 - Q, K are produced transposed: QT/KT [hd, t] (hd on partitions); scores are
   computed transposed as ST[k, q]; softmax runs WITHOUT max subtraction
   (scores are O(10), exp is safe in f32) and the row sums come for free as an
   extra ones-column appended to V in the P~V matmul.
 - V is produced in natural [t, hd] layout straight into VP [t, hd | 1].
 - P~ = exp(scale * ST) is masked only on diagonal 128x128 blocks; blocks
   entirely above the causal diagonal are never computed.
 - O = P~ @ V' lands as [q, hd | rowsum]; normalize by the reciprocal of the
   rowsum column, PE-transpose to OT [hd, q] for the final projection.

DMA queues: x tiles on SP/HWDGE (cold-start groups fanned across SP/Act/Pool —
the 16 HW DMA engines transfer in parallel, so issue bandwidth is what
matters), weights on Act/HWDGE in consumption order, mask/identity/v-bias and
the output stores on Pool/SWDGE (the last tile's stores on the by-then-idle SP
queue to shorten the drain).

Projections and the output projection run in fp8-e4m3 with
perf_mode=DoubleRow (two 128-contraction chunks packed per PE cell, ~1.44x+
over bf16 at FD=512). Weights are host-scaled x64 so they sit in the e4m3
normal range; Q/K biases carry the same x64 and the exp scale absorbs the
resulting 64^2 on the score path.

The V bias is never added on device: VP holds 64*(v - bv) = x@wv and the
softmax gives attn - bv exactly (Sum p (v-bv) / Sum p). That mean-subtraction
is what makes the fp8 output projection accurate: attn output is dominated by
the bv mean, and quantizing only the fluctuation part keeps the e4m3 noise
small. The ones-column is 4.0 (= 64/16) so the normalize yields 16*(attn-bv),
putting osb/ot in good e4m3 range; wo is x64 e4m3, so the stored f16 partial
is 1024x the true partial. The host divides by 1024 and adds bv@wo.T + bo in
f64. Scores and PV stay bf16 (PSUM accumulation always f32).
"""

import sys
import numpy as np

if '/opt/trn_rl_repo' not in sys.path:
    sys.path.insert(0, '/opt/trn_rl_repo')

import ml_dtypes
from contextlib import ExitStack

import concourse.mybir as mybir
import concourse.tile as tile
from concourse import bacc
from concourse.bass_utils import run_bass_kernel_spmd

B, T, D, H = 2, 2048, 2048, 16
HD = 128           # head dim
P = 128            # partitions
HPC = 4            # heads per core
NCORES = 8
SCALE = float(HD) ** -0.5
DC = D // P        # 16 contraction chunks for projections
NT = T // P        # 16 t-chunks of 128
QT_TILES = T // 512  # 4 t-tiles of 512

BF16 = mybir.dt.bfloat16
F32 = mybir.dt.float32
F16 = mybir.dt.float16
F8E4 = mybir.dt.float8e4
NPBF16 = ml_dtypes.bfloat16
NPF8E4 = ml_dtypes.float8_e4m3
WSCALE = 64.0          # host pre-scale on wq/wk/wv (+ biases) for fp8 range
DR = mybir.MatmulPerfMode.DoubleRow

_BUILD_CACHE = {}


def _merge(primary, secondary):
    """Interleave two thunk lists proportionally, preserving each list's
    internal order. Primary items sort first on ties."""
    items = []
    for li, lst in ((0, primary), (1, secondary)):
        n = len(lst)
        for i, fn in enumerate(lst):
            items.append(((i + 0.5) / n, li, i, fn))
    items.sort(key=lambda t: (t[0], t[1], t[2]))
    return [fn for _, _, _, fn in items]


def _weave(primary, secondary):
    for fn in _merge(primary, secondary):
        fn()


def _build(causal: bool):
    """Build the per-core Bass program (identical across cores; data differs)."""
    nc = bacc.Bacc("TRN2", target_bir_lowering=False, debug=False)

    xT = nc.dram_tensor("xT", [D, T], F8E4, kind="ExternalInput").ap()
    wqT = nc.dram_tensor("wqT", [D, HPC * HD], F8E4, kind="ExternalInput").ap()
    wkT = nc.dram_tensor("wkT", [D, HPC * HD], F8E4, kind="ExternalInput").ap()
    wvT = nc.dram_tensor("wvT", [D, HPC * HD], F8E4, kind="ExternalInput").ap()
    woT = nc.dram_tensor("woT", [HPC * HD, D], F8E4, kind="ExternalInput").ap()
    bq = nc.dram_tensor("bq", [P, HPC], F32, kind="ExternalInput").ap()
    bk = nc.dram_tensor("bk", [P, HPC], F32, kind="ExternalInput").ap()
    tri = nc.dram_tensor("tri", [P, P], BF16, kind="ExternalInput").ap()
    ident = nc.dram_tensor("ident", [P, P], BF16, kind="ExternalInput").ap()
    out = nc.dram_tensor("out", [T, D], F16, kind="ExternalOutput").ap()

    with tile.TileContext(nc) as tc:
        with ExitStack() as ctx:
            persist = ctx.enter_context(tc.tile_pool(name="persist", bufs=1))

            wq_sb = persist.tile([P, DC, HPC * HD], F8E4, name="wq_sb")
            wk_sb = persist.tile([P, DC, HPC * HD], F8E4, name="wk_sb")
            wv_sb = persist.tile([P, DC, HPC * HD], F8E4, name="wv_sb")
            wo_sb = persist.tile([P, HPC, D], F8E4, name="wo_sb")
            bq_sb = persist.tile([P, HPC], F32, name="bq_sb")
            bk_sb = persist.tile([P, HPC], F32, name="bk_sb")
            tri_sb = persist.tile([P, P], BF16, name="tri_sb")
            id_sb = persist.tile([P, P], BF16, name="id_sb")
            KT_sb = persist.tile([P, HPC, T], BF16, name="KT_sb")
            # V' with ones column: [t-within-chunk, head, t-chunk, hd+1]
            VP_sb = persist.tile([P, HPC, NT, HD + 1], BF16, name="VP_sb")

            # pools
            # 8 bufs: tile t4's 4 group tiles + all 4 prefetched t4+1 groups
            # may be alive at once — fewer bufs stalls the prefetch DMA on
            # pool reuse and the V chains then wait on x mid-tile
            ax = ctx.enter_context(tc.tile_pool(name="ax", bufs=8))
            aqt = ctx.enter_context(tc.tile_pool(name="aqt", bufs=2))
            aot = ctx.enter_context(tc.tile_pool(name="aot", bufs=3))
            cpt = ctx.enter_context(tc.tile_pool(name="cpt", bufs=48))
            csm = ctx.enter_context(tc.tile_pool(name="csm", bufs=8))
            cob = ctx.enter_context(tc.tile_pool(name="cob", bufs=6))
            ps = ctx.enter_context(tc.tile_pool(name="ps", bufs=1, space="PSUM"))

            def load_x(t4):
                """Group-DMAs of 4 chunks each on SP/HWDGE; returns per-PAIR
                AP views [P, 2, 512] indexed by dp (8 chunk-pairs)."""
                xas = []
                for g in range(4):
                    xg = ax.tile([P, 4, 512], F8E4, tag="xa", name=f"xa{t4}_{g}")
                    # tile 0 cold start: both HWDGE queues stream x in Q-chain
                    # consumption order (group 0 as 128KB pairs so the first
                    # matmul starts ASAP); the SW Pool queue is too slow here.
                    if t4 == 0 and g == 0:
                        for c in range(2):
                            nc.scalar.dma_start(
                                xg[:, 2 * c:2 * c + 2, :],
                                xT[256 * c:256 * (c + 1), 0:512].rearrange(
                                    "(c p) n -> p c n", p=P))
                    else:
                        if t4 == 0:
                            # g1 on the Act HW queue (behind g0's pairs);
                            # g2/g3 on Pool. Adding cold loads to the SP queue
                            # overflows its descriptor ring and the blocked
                            # issue op stalls wq -> 7us PE gap -> HAM rethrottle.
                            eng = nc.scalar if g == 1 else nc.gpsimd
                        else:
                            eng = nc.sync
                        eng.dma_start(
                            xg[:], xT[g * 512:(g + 1) * 512,
                                      t4 * 512:(t4 + 1) * 512].rearrange(
                                          "(c p) n -> p c n", p=P))
                    xas.extend(xg[:, 2 * c:2 * c + 2, :] for c in range(2))
                return xas

            NDP = DC // 2  # 8 fp8 DoubleRow chunk-pairs per projection

            def proj_items(t4, xas, qtile):
                """24 thunks: Q(8 d-pairs), K(8), V(8). Tag 'p' PSUM.
                All matmuls fp8-e4m3 perf_mode=DoubleRow (contraction 256)."""
                psq = [None] * HPC
                psk = [None] * HPC
                psv = [None] * HPC

                def q_chunk(dp):
                    def fn():
                        if dp == 0:
                            for h in range(HPC):
                                psq[h] = ps.tile([P, 512], F32, tag="p", bufs=4,
                                                 name=f"psq{t4}_{h}")
                        for h in range(HPC):
                            nc.tensor.matmul(psq[h][:],
                                             wq_sb[:, 2 * dp:2 * dp + 2, h * HD:(h + 1) * HD],
                                             xas[dp][:], start=(dp == 0),
                                             stop=(dp == NDP - 1), perf_mode=DR)
                        if dp == NDP - 1:
                            for h in range(HPC):
                                nc.vector.tensor_scalar_add(qtile[:, h, :], psq[h][:],
                                                            bq_sb[:, h:h + 1])
                    return fn

                def k_chunk(dp):
                    def fn():
                        if dp == 0:
                            for h in range(HPC):
                                psk[h] = ps.tile([P, 512], F32, tag="p", bufs=4,
                                                 name=f"psk{t4}_{h}")
                        for h in range(HPC):
                            nc.tensor.matmul(psk[h][:],
                                             wk_sb[:, 2 * dp:2 * dp + 2, h * HD:(h + 1) * HD],
                                             xas[dp][:], start=(dp == 0),
                                             stop=(dp == NDP - 1), perf_mode=DR)
                        if dp == NDP - 1:
                            for h in range(HPC):
                                nc.vector.tensor_scalar_add(
                                    KT_sb[:, h, t4 * 512:(t4 + 1) * 512], psk[h][:],
                                    bk_sb[:, h:h + 1])
                    return fn

                def v_half(j, half):
                    # V projection in natural [t, hd] layout: x chunk cols are
                    # the stationary operand, all 4 heads' weights move. One
                    # full chain per t-chunk j (not d-interleaved) so chunk
                    # j's PSUM bank frees early for the next tile's Q chains.
                    def fn():
                        if half == 0:
                            psv[j] = ps.tile([P, 512], F32, tag="p", bufs=4,
                                             name=f"psv{t4}_{j}")
                        for dp in range(4 * half, 4 * half + 4):
                            nc.tensor.matmul(psv[j][:],
                                             xas[dp][:, :, j * P:(j + 1) * P],
                                             wv_sb[:, 2 * dp:2 * dp + 2, :],
                                             start=(dp == 0), stop=(dp == NDP - 1),
                                             perf_mode=DR)
                        if half == 1:
                            kb = 4 * t4 + j
                            # no bias: VP holds 64*(v - bv) so the softmax
                            # output is mean-subtracted for the fp8 out-proj
                            for h in range(HPC):
                                nc.vector.tensor_copy(
                                    out=VP_sb[:, h, kb, 0:HD],
                                    in_=psv[j][:, h * HD:(h + 1) * HD])
                    return fn

                return ([q_chunk(dp) for dp in range(NDP)]
                        + [k_chunk(dp) for dp in range(NDP)]
                        + [v_half(j, half) for j in range(HPC) for half in (0, 1)])

            def make_attn(t4, qtile, ot_tile, pending_fins):
                """Attention for tile t4: per head scores->exp->mask, P~V
                chains with staggered PE transposes, and (after head 3) the
                output-projection partials + store. Head h+1's scores are
                emitted between head h's chains so the scalar engine's exps
                stay ahead of the PE."""
                kmax = 4 * t4 + 4 if causal else NT
                pts = [[None] * kmax for _ in range(HPC)]
                osbs = [[None] * HPC for _ in range(HPC)]

                def score_block(h, kb):
                    def fn():
                        qoff = max(0, kb - 4 * t4) * P if causal else 0
                        w = 512 - qoff
                        stp = ps.tile([P, 512], F32, tag="sf", bufs=2,
                                      name=f"st{t4}_{h}_{kb}")
                        nc.tensor.matmul(stp[:, 0:w], KT_sb[:, h, kb * P:(kb + 1) * P],
                                         qtile[:, h, qoff:512], start=True, stop=True)
                        pt = cpt.tile([P, 512], BF16, tag="pt", name=f"pt{t4}_{h}_{kb}")
                        nc.scalar.activation(pt[:, 0:w], stp[:, 0:w],
                                             mybir.ActivationFunctionType.Exp,
                                             scale=SCALE / (WSCALE * WSCALE))
                        if causal and kb >= 4 * t4:
                            nc.vector.tensor_mul(out=pt[:, 0:P], in0=pt[:, 0:P], in1=tri_sb[:])
                        pts[h][kb] = pt
                    return fn

                def pv_chain(h, qs):
                    def fn():
                        qb = 4 * t4 + qs
                        klim = qb + 1 if causal else NT
                        ops = ps.tile([P, HD + 1], F32, tag="o", bufs=2,
                                      name=f"o{t4}_{h}_{qs}")
                        for kb in range(klim):
                            qoff = max(0, kb - 4 * t4) * P if causal else 0
                            c0 = qs * P - qoff
                            nc.tensor.matmul(ops[:], pts[h][kb][:, c0:c0 + P],
                                             VP_sb[:, h, kb, :],
                                             start=(kb == 0), stop=(kb == klim - 1))
                        rec = csm.tile([P, 1], F32, tag="rec", name=f"rec{t4}_{h}_{qs}")
                        nc.vector.reciprocal(rec[:], ops[:, HD:HD + 1])
                        # ones-col is 4.0 = 64/16, so this is 16*(attn - bv):
                        # mean-subtracted and scaled into e4m3 range
                        osb = csm.tile([P, HD], BF16, tag="osb", name=f"osb{t4}_{h}_{qs}")
                        nc.vector.tensor_scalar_mul(osb[:], ops[:, 0:HD], rec[:])
                        osbs[h][qs] = osb
                    return fn

                def o_transpose(h, qs):
                    def fn():
                        tp2 = ps.tile([P, P], BF16, tag="o", bufs=2,
                                      name=f"tpo{t4}_{h}_{qs}")
                        nc.tensor.transpose(tp2[:], osbs[h][qs][:], id_sb[:])
                        # DVE cast bf16 -> e4m3 for the DoubleRow out-proj
                        nc.vector.tensor_copy(out=ot_tile[:, h, qs, :], in_=tp2[:])
                    return fn

                def fin(qs, n):
                    def fn():
                        tch = 4 * t4 + qs
                        # alternate PSUM tags so fins don't monopolize the
                        # score stream's two "sf" banks
                        fp = ps.tile([P, 512], F32, tag=("sf" if n % 2 else "o"),
                                     bufs=2, name=f"fin{t4}_{qs}_{n}")
                        for hh in range(0, HPC, 2):
                            nc.tensor.matmul(fp[:], ot_tile[:, hh:hh + 2, qs, :],
                                             wo_sb[:, hh:hh + 2, n * 512:(n + 1) * 512],
                                             start=(hh == 0), stop=(hh == HPC - 2),
                                             perf_mode=DR)
                        ob = cob.tile([P, 512], F16, tag="ob", name=f"ob{t4}_{qs}_{n}")
                        # last tile: exps are done, so the scalar engine is
                        # free — split the f32->f16 casts across DVE and ACT
                        # so the drain doesn't serialize on one engine
                        if t4 == QT_TILES - 1 and n % 2 == 0:
                            nc.scalar.copy(out=ob[:], in_=fp[:])
                        else:
                            nc.vector.tensor_copy(out=ob[:], in_=fp[:])
                        # all stores ride the SP HWDGE queue: the Pool SWDGE
                        # queue is too slow for 8MB of output (end-of-run
                        # backlog), and issuing from the Act engine steals
                        # issue slots from the exps. The sync engine is idle.
                        eng = nc.sync
                        eng.dma_start(out[tch * P:(tch + 1) * P,
                                          n * 512:(n + 1) * 512], ob[:])
                    return fn

                def pv_block(h):
                    # P~V chains with the transpose of chunk qs emitted after
                    # the NEXT chain so the PE never waits on the DVE
                    # normalize.
                    its = []
                    for qs in range(4):
                        its.append(pv_chain(h, qs))
                        if qs >= 1:
                            its.append(o_transpose(h, qs - 1))
                    its.append(o_transpose(h, 3))
                    return its

                # The tile's own output-projection partials are returned
                # separately and woven into the NEXT tile's Act-paced score
                # sections (where the PE would otherwise idle behind the
                # exps); the previous tile's partials arrive here as
                # pending_fins.
                #
                # Heads 0-1's OFF-DIAGONAL scores (kb < 4*t4, which need only
                # QT(t4) and older KT) are returned separately so they can run
                # inside tile t4's own K/V projection window — that pulls
                # ~14us of exp work off the exp-bound attention tail.
                off_heads = (0, 1) if causal and t4 > 0 else ()
                off_items = [score_block(h, kb)
                             for h in off_heads for kb in range(4 * t4)]
                fins = [fin(qs, n) for qs in range(4) for n in range(4)]
                nf = len(pending_fins)
                cuts = [0, nf * 1 // 10, nf * 2 // 10, nf * 6 // 10, nf]
                items = []
                for h in range(HPC):
                    k0 = 4 * t4 if h in off_heads else 0
                    filler = list(pending_fins[cuts[h]:cuts[h + 1]])
                    if h >= 1:
                        filler = _merge(pv_block(h - 1), filler)
                    items.extend(_merge(
                        [score_block(h, kb) for kb in range(k0, kmax)], filler))
                if t4 == QT_TILES - 1:
                    # last tile: the final head's transposes go right after
                    # their chains (PE briefly waits on the DVE normalize, but
                    # that frees fin(qs,*) immediately) and the fins follow so
                    # their casts/stores drain while later chains still run
                    h = HPC - 1
                    for qs in range(4):
                        items.append(pv_chain(h, qs))
                        items.append(o_transpose(h, qs))
                        items.extend(fins[4 * qs:4 * qs + 4])
                    fins = []
                else:
                    items.extend(pv_block(HPC - 1))
                return off_items, items, fins

            # ---- initial DMAs, spread across issue queues ----
            # HWDGE issue slots are the cold-start bottleneck (~1.25us per
            # DMA per queue), so bulk loads go as 4-chunk group DMAs via
            # einops views, split across the SP and Act queues with the
            # first Q matmul's dependencies (wq group 0 on SP, x group 0 on
            # Act) issued first on each.
            def wview(w, g):
                return w[g * 512:(g + 1) * 512, :].rearrange(
                    "(c p) n -> p c n", p=P)

            # first matmul needs only wq pair 0 + x pair 0: issue those as
            # 128KB pair-DMAs so they land ahead of the 3MB weight stream
            def wpair(w, p):
                return w[p * 256:(p + 1) * 256, :].rearrange(
                    "(c p) n -> p c n", p=P)

            nc.sync.dma_start(wq_sb[:, 0:2, :], wpair(wqT, 0))
            nc.sync.dma_start(wq_sb[:, 2:4, :], wpair(wqT, 1))
            xtiles = {0: load_x(0)}
            for g in range(1, 4):
                nc.sync.dma_start(wq_sb[:, 4 * g:4 * g + 4, :], wview(wqT, g))
            nc.sync.dma_start(bq_sb[:], bq[:])
            nc.sync.dma_start(bk_sb[:], bk[:])
            # non-critical loads go behind the x groups on the Pool queue so
            # they don't steal HBM bandwidth from the cold-start x/wq stream
            # (wk is first needed ~10us in, wv ~15us, wo ~45us)
            nc.gpsimd.dma_start(tri_sb[:], tri[:])
            nc.gpsimd.dma_start(id_sb[:], ident[:])
            for g in range(4):
                nc.gpsimd.dma_start(wk_sb[:, 4 * g:4 * g + 4, :], wview(wkT, g))
            for g in range(4):
                nc.gpsimd.dma_start(wv_sb[:, 4 * g:4 * g + 4, :], wview(wvT, g))
            for hh in range(HPC):
                nc.gpsimd.dma_start(wo_sb[:, hh, :], woT[hh * P:(hh + 1) * P, :])
            # ones column = 64/16: the rowsum keeps the V-path x64 scale down
            # to x4 so the normalize leaves x16 on (attn - bv) for e4m3 range
            nc.gpsimd.memset(VP_sb[:, :, :, HD:HD + 1], WSCALE / 16.0)

            # ---- main pipeline: attention(t4-1) weaves into proj(t4), and
            # the output-projection partials of t4-1 weave into attention(t4)
            # (PE filler for its Act-paced score warm-up) ----
            prev_attn, prev_fins = [], []
            for t4 in range(QT_TILES):
                qtile = aqt.tile([P, HPC, 512], BF16, tag="qt", name=f"qt{t4}")
                ot_tile = aot.tile([P, HPC, 4, P], F8E4, tag="ot", name=f"ot{t4}")
                pitems = proj_items(t4, xtiles[t4], qtile)
                if t4 + 1 < QT_TILES:
                    pitems.insert(0, (lambda n: (lambda: xtiles.__setitem__(
                        n, load_x(n))))(t4 + 1))
                off, items, fins = make_attn(t4, qtile, ot_tile, prev_fins)
                if off:
                    # off-diag scores of THIS tile need QT(t4): confine them
                    # to the K/V portion of the window
                    nq = NDP + (1 if t4 + 1 < QT_TILES else 0)
                    cut = len(prev_attn) * nq // len(pitems)
                    _weave(pitems[:nq], prev_attn[:cut])
                    _weave(pitems[nq:], _merge(prev_attn[cut:], off))
                else:
                    _weave(pitems, prev_attn)
                del xtiles[t4]
                prev_attn, prev_fins = items, fins
            for fn in prev_attn:
                fn()
            for fn in prev_fins:
                fn()

    nc.compile()
    return nc


def _get_program(causal: bool):
    if causal not in _BUILD_CACHE:
        _BUILD_CACHE[causal] = _build(causal)
    return _BUILD_CACHE[causal]


def _prep_in_maps(x, wq, bq, wk, bk, wv, bv, wo, bo):
    # x in e4m3 unscaled (|x| <~ 5.3, fp8 normals reach 2^-6; max 240).
    # Weights x64 so the uniform(+-0.038) range sits in e4m3 normals; the
    # matching x64 goes on the biases, is cancelled by the exp scale (Q,K)
    # and by the 64.0 ones-column (V).
    xf8 = np.asarray(x, dtype=np.float32).astype(NPF8E4)
    tri = np.triu(np.ones((P, P), dtype=np.float32)).astype(NPBF16)
    ident = np.eye(P, dtype=np.float32).astype(NPBF16)
    wqf8 = (np.asarray(wq, dtype=np.float32) * WSCALE).astype(NPF8E4)
    wkf8 = (np.asarray(wk, dtype=np.float32) * WSCALE).astype(NPF8E4)
    wvf8 = (np.asarray(wv, dtype=np.float32) * WSCALE).astype(NPF8E4)
    wof8 = (np.asarray(wo, dtype=np.float32) * WSCALE).astype(NPF8E4)

    in_maps = []
    for c in range(NCORES):
        b = c // 4
        hs = HPC * HD * (c % 4)
        sl = slice(hs, hs + HPC * HD)
        in_maps.append({
            "xT": np.ascontiguousarray(xf8[b].T),
            "wqT": np.ascontiguousarray(wqf8[sl, :].T),
            "wkT": np.ascontiguousarray(wkf8[sl, :].T),
            "wvT": np.ascontiguousarray(wvf8[sl, :].T),
            "woT": np.ascontiguousarray(wof8[:, sl].T),
            "bq": np.ascontiguousarray(
                (np.asarray(bq, np.float32) * WSCALE)[sl].reshape(HPC, P).T),
            "bk": np.ascontiguousarray(
                (np.asarray(bk, np.float32) * WSCALE)[sl].reshape(HPC, P).T),
            "tri": tri,
            "ident": ident,
        })
    return in_maps


def _classify_mask(mask):
    m = np.asarray(mask, dtype=np.float32).reshape(T, T)
    neg = np.isneginf(m)
    if not neg.any():
        return "full"
    if np.array_equal(neg, np.triu(np.ones((T, T), dtype=bool), k=1)):
        return "causal"
    return "other"


def _numpy_reference(x, mask, wq, bq, wk, bk, wv, bv, wo, bo):
    """Fallback for masks that are neither causal nor empty."""
    x = np.asarray(x, np.float32)
    m = np.asarray(mask, np.float32).reshape(T, T)
    q = (x.reshape(-1, D) @ np.asarray(wq, np.float32).T + bq).reshape(B, T, H, HD).transpose(0, 2, 1, 3)
    k = (x.reshape(-1, D) @ np.asarray(wk, np.float32).T + bk).reshape(B, T, H, HD).transpose(0, 2, 1, 3)
    v = (x.reshape(-1, D) @ np.asarray(wv, np.float32).T + bv).reshape(B, T, H, HD).transpose(0, 2, 1, 3)
    outh = np.empty((B, H, T, HD), np.float32)
    negm = np.isneginf(m)
    for b in range(B):
        for h in range(H):
            s = (q[b, h] @ k[b, h].T) * SCALE
            s = np.where(negm, -np.inf, s)
            s = s - s.max(axis=-1, keepdims=True)
            e = np.exp(s)
            p = e / e.sum(axis=-1, keepdims=True)
            outh[b, h] = p @ v[b, h]
    o = outh.transpose(0, 2, 1, 3).reshape(B * T, D)
    return (o @ np.asarray(wo, np.float32).T + bo).reshape(B, T, D).astype(np.float32)


def run_spmd(inputs, trace=False, tmpdir=None):
    """Run the device kernel; returns (output [B,T,D] f32, BassKernelResults)."""
    mode = _classify_mask(inputs["mask"])
    assert mode in ("causal", "full")
    nc = _get_program(mode == "causal")
    in_maps = _prep_in_maps(
        inputs["x"], inputs["wq"], inputs["bq"], inputs["wk"], inputs["bk"],
        inputs["wv"], inputs["bv"], inputs["wo"], inputs["bo"])
    kw = {}
    if trace:
        kw = dict(trace=True, tmpdir=tmpdir)
    # Unprofiled warm-up execution: the first run of a freshly-loaded NEFF
    # measures 5-60us slower (cold device caches); this also pre-populates
    # the jit cache so the measured run below is steady-state.
    try:
        from concourse import bass2jax
        bass2jax.run_bass_via_pjrt(nc, in_maps, n_cores=NCORES)
    except Exception:
        pass
    res = run_bass_kernel_spmd(nc, in_maps, core_ids=list(range(NCORES)), **kw)
    # device partials are 1024*((attn-bv) @ wo.T); add back the (constant)
    # mean row bv @ wo.T and bo here in f64
    bo64 = np.asarray(inputs["bo"], np.float64)
    mean64 = np.asarray(inputs["bv"], np.float64) @ np.asarray(
        inputs["wo"], np.float64).T + bo64
    out = np.empty((B, T, D), np.float32)
    for b in range(B):
        acc = np.zeros((T, D), np.float64)
        for c in range(4 * b, 4 * b + 4):
            acc += res.results[c]["out"].astype(np.float64)
        out[b] = (acc / 1024.0 + mean64).astype(np.float32)
    return out, res


def kernel(**inputs) -> np.ndarray:
    mode = _classify_mask(inputs["mask"])
    if mode == "other":
        return _numpy_reference(**inputs)
    out, _ = run_spmd(inputs)
    return out

